# revision 12
# baseline (speedup 1.0000x reference)
"""Trainium2 Bass kernel for CustomSelfAttention (B=8,S=1024,D=1024,H=16,K=64).

Strategy: data-parallel over batch across 8 NeuronCores (1 batch item/core).
Per-core pipeline (all matmuls in float32r = full-rate fp32 on the PE):
  0. x [S,D] -> PE-transpose -> xT [D,S]
  1. qT = (Wq/8)^T x^T, kT = Wk^T x^T   (layout [hk, s]);  v = x Wv ([s, hk])
     stored interleaved with a ones column per head: vext [s, h, 65]
  2. per head: scores^T [s,q] = kT_h^T-slice matmul qT_h; ACT exp with
     per-partition key-mask bias ((mask-1)*1e9) fused; ctx matmul with
     lhsT=[v_h | 1] gives ctx^T[k,q] plus the softmax row sums in one shot;
     normalize by 1/sums; DMA into ctxT [hk, q].
  3. out = ctxT^T Wo + bo, blended with the uniform-attention row for
     fully-masked queries (reference semantics: softmax of a row of -1e9 is
     exactly uniform), computed as u = mean_s(v) Wo.
"""

import sys

sys.path.insert(0, "/opt/trn_rl_repo")

import numpy as np  # noqa: E402

import concourse.bass as bass  # noqa: E402
import concourse.mybir as mybir  # noqa: E402
import concourse.tile as tile  # noqa: E402
from concourse import bacc  # noqa: E402
from concourse.bass_utils import run_bass_kernel_spmd  # noqa: E402
from concourse.masks import make_identity  # noqa: E402

F32 = mybir.dt.float32
F32R = mybir.dt.float32r
AF = mybir.ActivationFunctionType
OP = mybir.AluOpType

B, S, D, H, K = 8, 1024, 1024, 16, 64
HK = H * K
P = 128
SC = S // P      # 8 s-chunks
DC = D // P      # 8 d-chunks
HKC = HK // P    # 8 hk-chunks
NQW = S // 512   # 2 q-windows of 512
NEG = -1e9

TRACE = False  # set by test.py for profiling runs

_nc_cache = None


def _build_nc():
    nc = bacc.Bacc(None, target_bir_lowering=False)

    x_d = nc.dram_tensor("x", [S, D], F32, kind="ExternalInput")
    wq_d = nc.dram_tensor("wq", [D, HK], F32, kind="ExternalInput")
    wk_d = nc.dram_tensor("wk", [D, HK], F32, kind="ExternalInput")
    wv_d = nc.dram_tensor("wv", [D, HK], F32, kind="ExternalInput")
    wo_d = nc.dram_tensor("wo", [HK, D], F32, kind="ExternalInput")
    bq_d = nc.dram_tensor("bq", [HK], F32, kind="ExternalInput")
    bk_d = nc.dram_tensor("bk", [HK], F32, kind="ExternalInput")
    bv_d = nc.dram_tensor("bv", [HK], F32, kind="ExternalInput")
    bo_d = nc.dram_tensor("bo", [D], F32, kind="ExternalInput")
    ka_d = nc.dram_tensor("ka", [S], F32, kind="ExternalInput")   # (m-1)*1e9
    mq_d = nc.dram_tensor("mq", [S], F32, kind="ExternalInput")   # mask 0/1
    omq_d = nc.dram_tensor("omq", [S], F32, kind="ExternalInput")  # 1-mask
    ones_d = nc.dram_tensor("onec", [1], F32, kind="ExternalInput")
    out_d = nc.dram_tensor("out", [S, D], F32, kind="ExternalOutput")

    def bcast_ap(t, counts, step_last=None):
        # DRAM AP broadcasting a small tensor across leading 0-stride dims.
        # Innermost dim must be stride-1 for the DGE.
        ap = [[0, c] for c in counts]
        ap.append(step_last if step_last is not None else [1, 1])
        return bass.AP(tensor=t, offset=0, ap=ap)

    with tile.TileContext(nc) as tc:
        with (
            tc.tile_pool(name="consts", bufs=1) as consts,
            tc.tile_pool(name="big", bufs=1) as big,
            tc.tile_pool(name="share", bufs=1) as share,
            tc.tile_pool(name="xchunk", bufs=3) as xchunk,
            tc.tile_pool(name="wqk", bufs=3) as wqkp,
            tc.tile_pool(name="wbig", bufs=2) as wbigp,
            tc.tile_pool(name="epool", bufs=5) as epool,
            tc.tile_pool(name="rb", bufs=2) as rbpool,
            tc.tile_pool(name="rp", bufs=2) as rpool,
            tc.tile_pool(name="op", bufs=2) as opool,
            tc.tile_pool(name="dram", bufs=1, space="DRAM") as drampool,
            tc.tile_pool(name="pmm", bufs=4, space="PSUM") as pmm,
            tc.tile_pool(name="pctx", bufs=2, space="PSUM") as pctx,
            tc.tile_pool(name="psm", bufs=2, space="PSUM") as psm,
        ):
            # ---- constants ----
            ident = consts.tile([P, P], F32)
            make_identity(nc, ident[:])
            ka_sb = consts.tile([P, SC], F32)
            nc.sync.dma_start(ka_sb[:], ka_d.ap().rearrange("(c p) -> p c", p=P))
            mq_sb = consts.tile([P, SC], F32)
            nc.sync.dma_start(mq_sb[:], mq_d.ap().rearrange("(c p) -> p c", p=P))
            omq_sb = consts.tile([P, SC], F32)
            nc.sync.dma_start(omq_sb[:], omq_d.ap().rearrange("(c p) -> p c", p=P))
            bq_sb = consts.tile([P, HKC], F32)
            nc.sync.dma_start(bq_sb[:], bq_d.ap().rearrange("(c p) -> p c", p=P))
            bk_sb = consts.tile([P, HKC], F32)
            nc.sync.dma_start(bk_sb[:], bk_d.ap().rearrange("(c p) -> p c", p=P))
            bv_bc = consts.tile([P, HK], F32)
            nc.sync.dma_start(bv_bc[:], bcast_ap(bv_d, [P], [1, HK]))
            bo_bc = consts.tile([P, D], F32)
            nc.sync.dma_start(bo_bc[:], bcast_ap(bo_d, [P], [1, D]))
            ones_col = consts.tile([P, 1], F32R)
            nc.sync.dma_start(ones_col[:], bcast_ap(ones_d, [P]).bitcast(F32R))

            # ---- persistent big tensors ----
            # xT shares its SBUF slot with ctxT (xT dead after projections)
            xT = share.tile([P, DC * S], F32R, tag="share", name="xT").rearrange(
                "p (c s) -> p c s", c=DC
            )
            qT = big.tile([P, HKC, S], F32R, tag="qT")
            kT = big.tile([P, HKC, S], F32R, tag="kT")
            vext = big.tile([P, SC, H, K + 1], F32R, tag="vext")
            # ones column of vext via broadcast DMA (memset can't write f32r)
            nc.sync.dma_start(
                vext[:, :, :, K : K + 1].rearrange("p a b o -> p (a b) o"),
                bcast_ap(ones_d, [P, SC * H]).bitcast(F32R),
            )

            # ---- phase 0: transpose x -> xT ----
            for so in range(SC):
                for dhalf in range(2):
                    xc = xchunk.tile([P, 512], F32, tag="xc")
                    nc.sync.dma_start(
                        xc[:],
                        x_d.ap()[so * P : (so + 1) * P, dhalf * 512 : (dhalf + 1) * 512],
                    )
                    for dq in range(4):
                        dc = dhalf * 4 + dq
                        pt = pmm.tile([P, 512], F32, tag="mm")
                        nc.tensor.transpose(
                            pt[:, 0:P], xc[:, dq * P : (dq + 1) * P], ident[:]
                        )
                        nc.vector.tensor_copy(
                            xT[:, dc, so * P : (so + 1) * P], pt[:, 0:P]
                        )

            # ---- phase 1a: qT / kT projections ----
            for w_d, b_sb, dst in ((wq_d, bq_sb, qT), (wk_d, bk_sb, kT)):
                for hkc in range(HKC):
                    wts = []
                    for dhalf in range(2):
                        wt = wqkp.tile([P, 4, P], F32R, tag="wqk", name=f"wt{dhalf}")
                        nc.sync.dma_start(
                            wt[:],
                            w_d.ap()[
                                dhalf * 512 : (dhalf + 1) * 512,
                                hkc * P : (hkc + 1) * P,
                            ]
                            .rearrange("(c p) m -> p c m", p=P)
                            .bitcast(F32R),
                        )
                        wts.append(wt)
                    for qw in range(NQW):
                        ps = pmm.tile([P, 512], F32, tag="mm")
                        for dc in range(DC):
                            nc.tensor.matmul(
                                ps[:],
                                wts[dc // 4][:, dc % 4, :],
                                xT[:, dc, qw * 512 : (qw + 1) * 512],
                                start=(dc == 0),
                                stop=(dc == DC - 1),
                            )
                        nc.vector.tensor_scalar_add(
                            dst[:, hkc, qw * 512 : (qw + 1) * 512],
                            ps[:],
                            b_sb[:, hkc : hkc + 1],
                        )

            # ---- phase 1b: v projection into vext ----
            for hh in range(2):  # hk halves of 512
                wvt = wbigp.tile([P, DC, 512], F32R, tag="wbig")
                nc.sync.dma_start(
                    wvt[:],
                    wv_d.ap()[:, hh * 512 : (hh + 1) * 512]
                    .rearrange("(c p) n -> p c n", p=P)
                    .bitcast(F32R),
                )
                for st in range(SC):
                    ps = pmm.tile([P, 512], F32, tag="mm")
                    for dc in range(DC):
                        nc.tensor.matmul(
                            ps[:],
                            xT[:, dc, st * P : (st + 1) * P],
                            wvt[:, dc, :],
                            start=(dc == 0),
                            stop=(dc == DC - 1),
                        )
                    nc.vector.tensor_tensor(
                        vext[:, st, hh * 8 : (hh + 1) * 8, 0:K],
                        ps[:].rearrange("p (h k) -> p h k", k=K),
                        bv_bc[:, hh * 512 : (hh + 1) * 512].rearrange(
                            "p (h k) -> p h k", k=K
                        ),
                        OP.add,
                    )

            # ctxT reuses xT's SBUF slot (WAR handled by Tile)
            ctxT = share.tile(
                [P, HKC * S], F32R, tag="share", name="ctxT"
            ).rearrange("p (c s) -> p c s", c=HKC)

            # ---- phase 2: attention per head ----
            for h in range(H):
                hc, ho = h // 2, (h % 2) * 64
                for qw in range(NQW):
                    pc = pctx.tile([P, 512], F32, tag="ctx")
                    for sc in range(SC):
                        pss = pmm.tile([P, 512], F32, tag="mm")
                        nc.tensor.matmul(
                            pss[:],
                            kT[ho : ho + 64, hc, sc * P : (sc + 1) * P],
                            qT[ho : ho + 64, hc, qw * 512 : (qw + 1) * 512],
                            start=True,
                            stop=True,
                        )
                        ex = epool.tile([P, 512], F32R, tag="exp")
                        nc.scalar.activation(
                            ex[:], pss[:], AF.Exp, bias=ka_sb[:, sc : sc + 1],
                            scale=1.0,
                        )
                        nc.tensor.matmul(
                            pc[0:65, :],
                            vext[:, sc, h, :],
                            ex[:],
                            start=(sc == 0),
                            stop=(sc == SC - 1),
                        )
                    recip = rpool.tile([1, 512], F32, tag="rp")
                    nc.vector.reciprocal(recip[:], pc[64:65, :])
                    rb = rbpool.tile([64, 512], F32, tag="rb")
                    nc.gpsimd.partition_broadcast(rb[:], recip[:])
                    cn = epool.tile([64, 512], F32R, tag="exp", name="cn")
                    nc.vector.tensor_tensor(cn[:], pc[0:64, :], rb[:], OP.mult)
                    nc.sync.dma_start(
                        ctxT[ho : ho + 64, hc, qw * 512 : (qw + 1) * 512], cn[:]
                    )

            # ---- phase 3 prep: Wo + uniform-row fixup ----
            wot = []
            for dh in range(2):  # d halves
                w = wbigp.tile([P, HKC, 512], F32R, tag="wbig")
                nc.sync.dma_start(
                    w[:],
                    wo_d.ap()[:, dh * 512 : (dh + 1) * 512]
                    .rearrange("(c p) n -> p c n", p=P)
                    .bitcast(F32R),
                )
                wot.append(w)

            # mean_v [1, HK] = mean over s of v (incl. bias)
            mv_dram = drampool.tile([1, HK], F32)
            for hh in range(2):
                psu = psm.tile([P, 512], F32, tag="small")
                for sc in range(SC):
                    nc.tensor.matmul(
                        psu[0:1, :].rearrange("o (h k) -> o h k", k=K),
                        ones_col[:],
                        vext[:, sc, hh * 8 : (hh + 1) * 8, 0:K],
                        start=(sc == 0),
                        stop=(sc == SC - 1),
                    )
                mvh = rpool.tile([1, 512], F32, tag="rp")
                nc.vector.tensor_scalar_mul(mvh[:], psu[0:1, :], 1.0 / S)
                nc.sync.dma_start(mv_dram[0:1, hh * 512 : (hh + 1) * 512], mvh[:])
            mvT = consts.tile([P, HKC], F32R)
            nc.sync.dma_start(
                mvT[:],
                mv_dram[:].rearrange("o (c p) -> (o p) c", p=P).bitcast(F32R),
            )
            # u [1, D] = mean_v @ Wo, broadcast per half
            u_bc = consts.tile([P, D], F32)
            for dh in range(2):
                psu = psm.tile([P, 512], F32, tag="small")
                for c in range(HKC):
                    nc.tensor.matmul(
                        psu[0:1, :],
                        mvT[:, c : c + 1],
                        wot[dh][:, c, :],
                        start=(c == 0),
                        stop=(c == HKC - 1),
                    )
                uh = rpool.tile([1, 512], F32, tag="rp")
                nc.vector.tensor_copy(uh[:], psu[0:1, :])
                nc.gpsimd.partition_broadcast(
                    u_bc[:, dh * 512 : (dh + 1) * 512], uh[:]
                )
            # u_bc <- u + bo (the blend adds mq*(out - u) on top)
            nc.vector.tensor_tensor(u_bc[:], u_bc[:], bo_bc[:], OP.add)

            # ---- phase 3: output projection + blend ----
            for qt in range(SC):  # q-chunks of 128
                for dh in range(2):
                    po = psm.tile([P, 512], F32, tag="small")
                    for c in range(HKC):
                        nc.tensor.matmul(
                            po[:],
                            ctxT[:, c, qt * P : (qt + 1) * P],
                            wot[dh][:, c, :],
                            start=(c == 0),
                            stop=(c == HKC - 1),
                        )
                    # out = (po - (u+bo))*mq + (u+bo)
                    #     = mq*po + (1-mq)*u + bo   (since mq*bo+(1-mq)*bo=bo)
                    ub = u_bc[:, dh * 512 : (dh + 1) * 512]
                    t1 = opool.tile([P, 512], F32, tag="o1")
                    nc.vector.tensor_tensor(t1[:], po[:], ub, OP.subtract)
                    nc.vector.scalar_tensor_tensor(
                        t1[:], t1[:], mq_sb[:, qt : qt + 1], ub, OP.mult, OP.add
                    )
                    nc.sync.dma_start(
                        out_d.ap()[
                            qt * P : (qt + 1) * P, dh * 512 : (dh + 1) * 512
                        ],
                        t1[:],
                    )

    nc.compile()
    return nc


def _get_nc():
    global _nc_cache
    if _nc_cache is None:
        _nc_cache = _build_nc()
    return _nc_cache


def _make_in_maps(np_inputs):
    return _prep_in_maps(**np_inputs)


def _prep_in_maps(input_tensor, input_mask, Wq, bq, Wk, bk, Wv, bv, Wo, bo):
    x = np.ascontiguousarray(np.asarray(input_tensor, dtype=np.float32))
    mask = np.asarray(input_mask).astype(bool)
    Wq = np.asarray(Wq, dtype=np.float32).reshape(D, HK)
    Wk = np.asarray(Wk, dtype=np.float32).reshape(D, HK)
    Wv = np.asarray(Wv, dtype=np.float32).reshape(D, HK)
    Wo = np.asarray(Wo, dtype=np.float32).reshape(HK, D)
    bq = np.asarray(bq, dtype=np.float32).reshape(HK)
    bk = np.asarray(bk, dtype=np.float32).reshape(HK)
    bv = np.asarray(bv, dtype=np.float32).reshape(HK)
    bo = np.asarray(bo, dtype=np.float32).reshape(D)

    # fold the 1/sqrt(K)=1/8 score scale into Wq/bq (exact: power of two)
    wqs = np.ascontiguousarray(Wq / 8.0)
    bqs = bq / 8.0

    mf = mask.astype(np.float32)
    ka = (mf - 1.0) * 1e9   # 0 where kept, -1e9 where masked
    omq = 1.0 - mf
    onec = np.ones(1, np.float32)

    in_maps = []
    for b in range(B):
        in_maps.append(
            {
                "x": np.ascontiguousarray(x[b]),
                "wq": wqs,
                "wk": np.ascontiguousarray(Wk),
                "wv": np.ascontiguousarray(Wv),
                "wo": np.ascontiguousarray(Wo),
                "bq": np.ascontiguousarray(bqs),
                "bk": np.ascontiguousarray(bk),
                "bv": np.ascontiguousarray(bv),
                "bo": np.ascontiguousarray(bo),
                "ka": np.ascontiguousarray(ka[b]),
                "mq": np.ascontiguousarray(mf[b]),
                "omq": np.ascontiguousarray(omq[b]),
                "onec": onec,
            }
        )
    return in_maps


def kernel(input_tensor, input_mask, Wq, bq, Wk, bk, Wv, bv, Wo, bo):
    in_maps = _prep_in_maps(
        input_tensor, input_mask, Wq, bq, Wk, bk, Wv, bv, Wo, bo
    )
    nc = _get_nc()
    res = run_bass_kernel_spmd(nc, in_maps, core_ids=list(range(B)), trace=TRACE)
    if TRACE:
        kernel.last_result = res
    out = np.stack([r["out"] for r in res.results], axis=0)
    return out


# revision 14
# speedup vs baseline: 24.7280x; 24.7280x over previous
"""Trainium2 Bass kernel for CustomSelfAttention (B=8,S=1024,D=1024,H=16,K=64).

Strategy: data-parallel over batch across 8 NeuronCores (1 batch item/core).
Per-core pipeline (all matmuls in float32r = full-rate fp32 on the PE):
  0. x [S,D] -> PE-transpose -> xT [D,S]
  1. qT = (Wq/8)^T x^T, kT = Wk^T x^T   (layout [hk, s]);  v = x Wv ([s, hk])
     stored interleaved with a ones column per head: vext [s, h, 65]
  2. per head: scores^T [s,q] = kT_h^T-slice matmul qT_h; ACT exp with
     per-partition key-mask bias ((mask-1)*1e9) fused; ctx matmul with
     lhsT=[v_h | 1] gives ctx^T[k,q] plus the softmax row sums in one shot;
     normalize by 1/sums; DMA into ctxT [hk, q].
  3. out = ctxT^T Wo + bo, blended with the uniform-attention row for
     fully-masked queries (reference semantics: softmax of a row of -1e9 is
     exactly uniform), computed as u = mean_s(v) Wo.
"""

import contextlib
import sys

sys.path.insert(0, "/opt/trn_rl_repo")

import numpy as np  # noqa: E402

import concourse.bass as bass  # noqa: E402
import concourse.mybir as mybir  # noqa: E402
import concourse.tile as tile  # noqa: E402
from concourse import bacc  # noqa: E402
from concourse.bass_utils import run_bass_kernel_spmd  # noqa: E402
from concourse.masks import make_identity  # noqa: E402

F32 = mybir.dt.float32
F32R = mybir.dt.float32r
AF = mybir.ActivationFunctionType
OP = mybir.AluOpType

B, S, D, H, K = 8, 1024, 1024, 16, 64
HK = H * K
P = 128
SC = S // P      # 8 s-chunks
DC = D // P      # 8 d-chunks
HKC = HK // P    # 8 hk-chunks
NQW = S // 512   # 2 q-windows of 512
NEG = -1e9

TRACE = False  # set by test.py for profiling runs

_nc_cache = None


def _build_nc(repeat=1):
    nc = bacc.Bacc(None, target_bir_lowering=False)

    x_d = nc.dram_tensor("x", [S, D], F32, kind="ExternalInput")
    wq_d = nc.dram_tensor("wq", [D, HK], F32, kind="ExternalInput")
    wk_d = nc.dram_tensor("wk", [D, HK], F32, kind="ExternalInput")
    wv_d = nc.dram_tensor("wv", [D, HK], F32, kind="ExternalInput")
    wo_d = nc.dram_tensor("wo", [HK, D], F32, kind="ExternalInput")
    bq_d = nc.dram_tensor("bq", [HK], F32, kind="ExternalInput")
    bk_d = nc.dram_tensor("bk", [HK], F32, kind="ExternalInput")
    bv_d = nc.dram_tensor("bv", [HK], F32, kind="ExternalInput")
    bo_d = nc.dram_tensor("bo", [D], F32, kind="ExternalInput")
    ka_d = nc.dram_tensor("ka", [S], F32, kind="ExternalInput")   # (m-1)*1e9
    mq_d = nc.dram_tensor("mq", [S], F32, kind="ExternalInput")   # mask 0/1
    omq_d = nc.dram_tensor("omq", [S], F32, kind="ExternalInput")  # 1-mask
    ones_d = nc.dram_tensor("onec", [1], F32, kind="ExternalInput")
    out_d = nc.dram_tensor("out", [S, D], F32, kind="ExternalOutput")

    def bcast_ap(t, counts, step_last=None):
        # DRAM AP broadcasting a small tensor across leading 0-stride dims.
        # Innermost dim must be stride-1 for the DGE.
        ap = [[0, c] for c in counts]
        ap.append(step_last if step_last is not None else [1, 1])
        return bass.AP(tensor=t, offset=0, ap=ap)

    with tile.TileContext(nc) as tc:
        with (
            tc.tile_pool(name="consts", bufs=1) as consts,
            tc.tile_pool(name="big", bufs=1) as big,
            tc.tile_pool(name="share", bufs=1) as share,
            tc.tile_pool(name="xchunk", bufs=3) as xchunk,
            tc.tile_pool(name="wqk", bufs=3) as wqkp,
            tc.tile_pool(name="wbig", bufs=2) as wbigp,
            tc.tile_pool(name="epool", bufs=5) as epool,
            tc.tile_pool(name="rb", bufs=2) as rbpool,
            tc.tile_pool(name="rp", bufs=2) as rpool,
            tc.tile_pool(name="op", bufs=2) as opool,
            tc.tile_pool(name="dram", bufs=1, space="DRAM") as drampool,
            tc.tile_pool(name="pmm", bufs=4, space="PSUM") as pmm,
            tc.tile_pool(name="pctx", bufs=2, space="PSUM") as pctx,
            tc.tile_pool(name="psm", bufs=2, space="PSUM") as psm,
        ):
            # ---- constants ----
            ident = consts.tile([P, P], F32)
            make_identity(nc, ident[:])
            ka_sb = consts.tile([P, SC], F32)
            nc.sync.dma_start(ka_sb[:], ka_d.ap().rearrange("(c p) -> p c", p=P))
            mq_sb = consts.tile([P, SC], F32)
            nc.sync.dma_start(mq_sb[:], mq_d.ap().rearrange("(c p) -> p c", p=P))
            omq_sb = consts.tile([P, SC], F32)
            nc.sync.dma_start(omq_sb[:], omq_d.ap().rearrange("(c p) -> p c", p=P))
            bq_sb = consts.tile([P, HKC], F32)
            nc.sync.dma_start(bq_sb[:], bq_d.ap().rearrange("(c p) -> p c", p=P))
            bk_sb = consts.tile([P, HKC], F32)
            nc.sync.dma_start(bk_sb[:], bk_d.ap().rearrange("(c p) -> p c", p=P))
            bv_bc = consts.tile([P, HK], F32)
            nc.sync.dma_start(bv_bc[:], bcast_ap(bv_d, [P], [1, HK]))
            bo_bc = consts.tile([P, D], F32)
            nc.sync.dma_start(bo_bc[:], bcast_ap(bo_d, [P], [1, D]))
            ones_col = consts.tile([P, 1], F32R)
            nc.sync.dma_start(ones_col[:], bcast_ap(ones_d, [P]).bitcast(F32R))

            # Optional on-device repeat loop for benchmarking: amortizes the
            # huge per-call axon tunnel overhead across `repeat` executions.
            loop_cm = (
                tc.For_i(
                    0,
                    repeat,
                    1,
                    hint_engines=(
                        mybir.EngineType.PE,
                        mybir.EngineType.Activation,
                        mybir.EngineType.DVE,
                        mybir.EngineType.SP,
                        mybir.EngineType.Pool,
                    ),
                )
                if repeat > 1
                else contextlib.nullcontext()
            )
            with loop_cm:
                _emit_body(
                    nc, tc, x_d, wq_d, wk_d, wv_d, wo_d, out_d, bcast_ap,
                    ones_d, ident, ka_sb, mq_sb, omq_sb, bq_sb, bk_sb, bv_bc,
                    bo_bc, ones_col, consts, big, share, xchunk, wqkp, wbigp,
                    epool, rbpool, rpool, opool, drampool, pmm, pctx, psm,
                )

    nc.compile()
    return nc


def _emit_body(
    nc, tc, x_d, wq_d, wk_d, wv_d, wo_d, out_d, bcast_ap, ones_d, ident,
    ka_sb, mq_sb, omq_sb, bq_sb, bk_sb, bv_bc, bo_bc, ones_col, consts, big,
    share, xchunk, wqkp, wbigp, epool, rbpool, rpool, opool, drampool, pmm,
    pctx, psm,
):
    if True:
        if True:
            # ---- persistent big tensors ----
            # xT shares its SBUF slot with ctxT (xT dead after projections)
            xT = share.tile([P, DC * S], F32R, tag="share", name="xT").rearrange(
                "p (c s) -> p c s", c=DC
            )
            qT = big.tile([P, HKC, S], F32R, tag="qT")
            kT = big.tile([P, HKC, S], F32R, tag="kT")
            vext = big.tile([P, SC, H, K + 1], F32R, tag="vext")
            # ones column of vext via broadcast DMA (memset can't write f32r)
            nc.sync.dma_start(
                vext[:, :, :, K : K + 1].rearrange("p a b o -> p (a b) o"),
                bcast_ap(ones_d, [P, SC * H]).bitcast(F32R),
            )

            # ---- phase 0: transpose x -> xT ----
            for so in range(SC):
                for dhalf in range(2):
                    xc = xchunk.tile([P, 512], F32, tag="xc")
                    nc.sync.dma_start(
                        xc[:],
                        x_d.ap()[so * P : (so + 1) * P, dhalf * 512 : (dhalf + 1) * 512],
                    )
                    for dq in range(4):
                        dc = dhalf * 4 + dq
                        pt = pmm.tile([P, 512], F32, tag="mm")
                        nc.tensor.transpose(
                            pt[:, 0:P], xc[:, dq * P : (dq + 1) * P], ident[:]
                        )
                        nc.vector.tensor_copy(
                            xT[:, dc, so * P : (so + 1) * P], pt[:, 0:P]
                        )

            # ---- phase 1a: qT / kT projections ----
            for w_d, b_sb, dst in ((wq_d, bq_sb, qT), (wk_d, bk_sb, kT)):
                for hkc in range(HKC):
                    wts = []
                    for dhalf in range(2):
                        wt = wqkp.tile([P, 4, P], F32R, tag="wqk", name=f"wt{dhalf}")
                        nc.sync.dma_start(
                            wt[:],
                            w_d.ap()[
                                dhalf * 512 : (dhalf + 1) * 512,
                                hkc * P : (hkc + 1) * P,
                            ]
                            .rearrange("(c p) m -> p c m", p=P)
                            .bitcast(F32R),
                        )
                        wts.append(wt)
                    for qw in range(NQW):
                        ps = pmm.tile([P, 512], F32, tag="mm")
                        for dc in range(DC):
                            nc.tensor.matmul(
                                ps[:],
                                wts[dc // 4][:, dc % 4, :],
                                xT[:, dc, qw * 512 : (qw + 1) * 512],
                                start=(dc == 0),
                                stop=(dc == DC - 1),
                            )
                        nc.vector.tensor_scalar_add(
                            dst[:, hkc, qw * 512 : (qw + 1) * 512],
                            ps[:],
                            b_sb[:, hkc : hkc + 1],
                        )

            # ---- phase 1b: v projection into vext ----
            for hh in range(2):  # hk halves of 512
                wvt = wbigp.tile([P, DC, 512], F32R, tag="wbig")
                nc.sync.dma_start(
                    wvt[:],
                    wv_d.ap()[:, hh * 512 : (hh + 1) * 512]
                    .rearrange("(c p) n -> p c n", p=P)
                    .bitcast(F32R),
                )
                for st in range(SC):
                    ps = pmm.tile([P, 512], F32, tag="mm")
                    for dc in range(DC):
                        nc.tensor.matmul(
                            ps[:],
                            xT[:, dc, st * P : (st + 1) * P],
                            wvt[:, dc, :],
                            start=(dc == 0),
                            stop=(dc == DC - 1),
                        )
                    nc.vector.tensor_tensor(
                        vext[:, st, hh * 8 : (hh + 1) * 8, 0:K],
                        ps[:].rearrange("p (h k) -> p h k", k=K),
                        bv_bc[:, hh * 512 : (hh + 1) * 512].rearrange(
                            "p (h k) -> p h k", k=K
                        ),
                        OP.add,
                    )

            # ctxT reuses xT's SBUF slot (WAR handled by Tile)
            ctxT = share.tile(
                [P, HKC * S], F32R, tag="share", name="ctxT"
            ).rearrange("p (c s) -> p c s", c=HKC)

            # ---- phase 2: attention per head ----
            for h in range(H):
                hc, ho = h // 2, (h % 2) * 64
                for qw in range(NQW):
                    pc = pctx.tile([P, 512], F32, tag="ctx")
                    for sc in range(SC):
                        pss = pmm.tile([P, 512], F32, tag="mm")
                        nc.tensor.matmul(
                            pss[:],
                            kT[ho : ho + 64, hc, sc * P : (sc + 1) * P],
                            qT[ho : ho + 64, hc, qw * 512 : (qw + 1) * 512],
                            start=True,
                            stop=True,
                        )
                        ex = epool.tile([P, 512], F32R, tag="exp")
                        nc.scalar.activation(
                            ex[:], pss[:], AF.Exp, bias=ka_sb[:, sc : sc + 1],
                            scale=1.0,
                        )
                        nc.tensor.matmul(
                            pc[0:65, :],
                            vext[:, sc, h, :],
                            ex[:],
                            start=(sc == 0),
                            stop=(sc == SC - 1),
                        )
                    recip = rpool.tile([1, 512], F32, tag="rp")
                    nc.vector.reciprocal(recip[:], pc[64:65, :])
                    rb = rbpool.tile([64, 512], F32, tag="rb")
                    nc.gpsimd.partition_broadcast(rb[:], recip[:])
                    cn = epool.tile([64, 512], F32R, tag="exp", name="cn")
                    nc.vector.tensor_tensor(cn[:], pc[0:64, :], rb[:], OP.mult)
                    nc.sync.dma_start(
                        ctxT[ho : ho + 64, hc, qw * 512 : (qw + 1) * 512], cn[:]
                    )

            # ---- phase 3 prep: Wo + uniform-row fixup ----
            wot = []
            for dh in range(2):  # d halves
                w = wbigp.tile([P, HKC, 512], F32R, tag="wbig")
                nc.sync.dma_start(
                    w[:],
                    wo_d.ap()[:, dh * 512 : (dh + 1) * 512]
                    .rearrange("(c p) n -> p c n", p=P)
                    .bitcast(F32R),
                )
                wot.append(w)

            # mean_v [1, HK] = mean over s of v (incl. bias)
            mv_dram = drampool.tile([1, HK], F32)
            for hh in range(2):
                psu = psm.tile([P, 512], F32, tag="small")
                for sc in range(SC):
                    nc.tensor.matmul(
                        psu[0:1, :].rearrange("o (h k) -> o h k", k=K),
                        ones_col[:],
                        vext[:, sc, hh * 8 : (hh + 1) * 8, 0:K],
                        start=(sc == 0),
                        stop=(sc == SC - 1),
                    )
                mvh = rpool.tile([1, 512], F32, tag="rp")
                nc.vector.tensor_scalar_mul(mvh[:], psu[0:1, :], 1.0 / S)
                nc.sync.dma_start(mv_dram[0:1, hh * 512 : (hh + 1) * 512], mvh[:])
            mvT = consts.tile([P, HKC], F32R)
            nc.sync.dma_start(
                mvT[:],
                mv_dram[:].rearrange("o (c p) -> (o p) c", p=P).bitcast(F32R),
            )
            # u [1, D] = mean_v @ Wo, broadcast per half
            u_bc = consts.tile([P, D], F32)
            for dh in range(2):
                psu = psm.tile([P, 512], F32, tag="small")
                for c in range(HKC):
                    nc.tensor.matmul(
                        psu[0:1, :],
                        mvT[:, c : c + 1],
                        wot[dh][:, c, :],
                        start=(c == 0),
                        stop=(c == HKC - 1),
                    )
                uh = rpool.tile([1, 512], F32, tag="rp")
                nc.vector.tensor_copy(uh[:], psu[0:1, :])
                nc.gpsimd.partition_broadcast(
                    u_bc[:, dh * 512 : (dh + 1) * 512], uh[:]
                )
            # u_bc <- u + bo (the blend adds mq*(out - u) on top)
            nc.vector.tensor_tensor(u_bc[:], u_bc[:], bo_bc[:], OP.add)

            # ---- phase 3: output projection + blend ----
            for qt in range(SC):  # q-chunks of 128
                for dh in range(2):
                    po = psm.tile([P, 512], F32, tag="small")
                    for c in range(HKC):
                        nc.tensor.matmul(
                            po[:],
                            ctxT[:, c, qt * P : (qt + 1) * P],
                            wot[dh][:, c, :],
                            start=(c == 0),
                            stop=(c == HKC - 1),
                        )
                    # out = (po - (u+bo))*mq + (u+bo)
                    #     = mq*po + (1-mq)*u + bo   (since mq*bo+(1-mq)*bo=bo)
                    ub = u_bc[:, dh * 512 : (dh + 1) * 512]
                    t1 = opool.tile([P, 512], F32, tag="o1")
                    nc.vector.tensor_tensor(t1[:], po[:], ub, OP.subtract)
                    nc.vector.scalar_tensor_tensor(
                        t1[:], t1[:], mq_sb[:, qt : qt + 1], ub, OP.mult, OP.add
                    )
                    nc.sync.dma_start(
                        out_d.ap()[
                            qt * P : (qt + 1) * P, dh * 512 : (dh + 1) * 512
                        ],
                        t1[:],
                    )


def _get_nc():
    global _nc_cache
    if _nc_cache is None:
        _nc_cache = _build_nc()
    return _nc_cache


_nc_bench_cache = {}


def _get_bench_nc(repeat):
    if repeat not in _nc_bench_cache:
        _nc_bench_cache[repeat] = _build_nc(repeat)
    return _nc_bench_cache[repeat]


def _make_in_maps(np_inputs):
    return _prep_in_maps(**np_inputs)


def _prep_in_maps(input_tensor, input_mask, Wq, bq, Wk, bk, Wv, bv, Wo, bo):
    x = np.ascontiguousarray(np.asarray(input_tensor, dtype=np.float32))
    mask = np.asarray(input_mask).astype(bool)
    Wq = np.asarray(Wq, dtype=np.float32).reshape(D, HK)
    Wk = np.asarray(Wk, dtype=np.float32).reshape(D, HK)
    Wv = np.asarray(Wv, dtype=np.float32).reshape(D, HK)
    Wo = np.asarray(Wo, dtype=np.float32).reshape(HK, D)
    bq = np.asarray(bq, dtype=np.float32).reshape(HK)
    bk = np.asarray(bk, dtype=np.float32).reshape(HK)
    bv = np.asarray(bv, dtype=np.float32).reshape(HK)
    bo = np.asarray(bo, dtype=np.float32).reshape(D)

    # fold the 1/sqrt(K)=1/8 score scale into Wq/bq (exact: power of two)
    wqs = np.ascontiguousarray(Wq / 8.0)
    bqs = bq / 8.0

    mf = mask.astype(np.float32)
    ka = (mf - 1.0) * 1e9   # 0 where kept, -1e9 where masked
    omq = 1.0 - mf
    onec = np.ones(1, np.float32)

    in_maps = []
    for b in range(B):
        in_maps.append(
            {
                "x": np.ascontiguousarray(x[b]),
                "wq": wqs,
                "wk": np.ascontiguousarray(Wk),
                "wv": np.ascontiguousarray(Wv),
                "wo": np.ascontiguousarray(Wo),
                "bq": np.ascontiguousarray(bqs),
                "bk": np.ascontiguousarray(bk),
                "bv": np.ascontiguousarray(bv),
                "bo": np.ascontiguousarray(bo),
                "ka": np.ascontiguousarray(ka[b]),
                "mq": np.ascontiguousarray(mf[b]),
                "omq": np.ascontiguousarray(omq[b]),
                "onec": onec,
            }
        )
    return in_maps


def kernel(input_tensor, input_mask, Wq, bq, Wk, bk, Wv, bv, Wo, bo):
    in_maps = _prep_in_maps(
        input_tensor, input_mask, Wq, bq, Wk, bk, Wv, bv, Wo, bo
    )
    nc = _get_nc()
    res = run_bass_kernel_spmd(nc, in_maps, core_ids=list(range(B)), trace=TRACE)
    if TRACE:
        kernel.last_result = res
    out = np.stack([r["out"] for r in res.results], axis=0)
    return out
